# revision 29
# baseline (speedup 1.0000x reference)
"""Chunked sigmoid MHA on 8 Trainium2 NeuronCores (Bass/Tile).

Problem: out = (sigmoid(scale * (x_q Wq^T)(x_k Wk^T)^T) @ (x_v Wv^T)) @ Wo^T
with B=2, L=S=2048, E=1024, H=16, D=64.

Sharding: (batch, head-group) - core c handles batch b=c//4 and heads
[4g, 4g+4) with g=c%4.  Each core computes its 4 heads' Q/K/V projections
(column slices of Wq/Wk/Wv), full sigmoid attention for those heads, and a
partial output projection (row slice of Wo^T); the host sums the 4 partial
outputs per batch.

v2 layout/schedule:
  - Host pre-arranges x inputs chunk-major ([128, 4*4096]: chunk c of 512
    seq positions, within chunk e-major) so each chunk loads with one
    dma_start of 128 x 8KB descriptors (near line rate), and weights as a
    single [128, 8192] tensor (wk|wq|wv|wo, e/m-chunk-major).
  - DMA priority: three issue rings (sync/scalar HWDGE, gpsimd SWDGE)
    ordered by deadline: xk c0 + xq c0 + wk/wq first (k/q proj chunk 0),
    then xv c0/wv, then later k chunks, with q-proj chunks 1-3 last
    (their scores only run at lc>=1).
  - ACT sigmoid table preloaded at t~0 with a dummy activation.
  - Emission pipeline: scores+sigmoid run ~2 pairs ahead of the
    attention-output stream; projection chunks / output projection are
    interleaved as filler to keep the PE busy during sigmoid latency.
  - Output stored as [128, 1024] f32 blocks (one dma_start each, 4KB
    descriptors).
"""

import ml_dtypes
import numpy as np

import concourse.bass as bass
import concourse.mybir as mybir
import concourse.tile as tile
from concourse import bass_utils
from concourse.vector_clock import ScopedClock

F32 = mybir.dt.float32
BF16 = mybir.dt.bfloat16
AF = mybir.ActivationFunctionType

E = 1024          # embed dim
L = 2048          # sequence length (queries == keys)
DH = 256          # per-core projection dim (4 heads x 64)
EC = E // 128     # 8 E-chunks of 128
LC = L // 512     # 4 L-chunks of 512
ST = L // 128     # 16 S-tiles of 128
CW = 8 * 512      # 4096 cols per x chunk tile
SCALE = 64 ** -0.5  # 0.125, applied inside the sigmoid activation

WK_OFF = 0
WQ_OFF = 2048
WV_OFF = 4096
WO_OFF = 6144

N_CORES = 8


class SplitDrainTileContext(tile.TileContext):
    """This walrus build rejects >1 sync wait on the SP CTRL (Drain)
    instruction, and Tile's end-of-kernel drain waits on every used proc.
    Split the waits across a chain of single-wait drains."""

    DRAIN_WAIT_CAP = 1

    def _drain_and_barrier(self, tick_clock, wait_clock):
        nc = self.nc
        drain_inst = nc.sync.drain()
        wait_clock.add_sem_waits(
            drain_inst.ins, ScopedClock({None: tick_clock.global_clock})
        )
        si = drain_inst.ins.sync_info
        waits = list(si.on_wait) if si is not None else []
        if len(waits) > self.DRAIN_WAIT_CAP:
            si.on_wait = waits[: self.DRAIN_WAIT_CAP]
            for i in range(self.DRAIN_WAIT_CAP, len(waits), self.DRAIN_WAIT_CAP):
                extra = nc.sync.drain()
                esi = extra.ins.sync_info
                if esi is None:
                    esi = mybir.SyncInfo(on_wait=[], on_update=[])
                esi.on_wait = waits[i : i + self.DRAIN_WAIT_CAP]
                extra.ins.sync_info = esi
        nc.all_engine_barrier()
        assert self.sems is not None
        popped = nc._tile_sem_poison_stack.pop()
        assert popped is self._sem_poison
        nc.clear_and_free_semaphores(list(self.sems.allocated().values()))
        nc.all_engine_barrier()


def build_nc() -> bass.Bass:
    nc = bass.Bass("TRN2", target_bir_lowering=False, debug=False)

    xq = nc.dram_tensor("xq", [128, 4 * CW], BF16, kind="ExternalInput").ap()
    xk = nc.dram_tensor("xk", [128, 4 * CW], BF16, kind="ExternalInput").ap()
    xv = nc.dram_tensor("xv", [128, 4 * CW], BF16, kind="ExternalInput").ap()
    wall = nc.dram_tensor("wall", [128, 8192], BF16, kind="ExternalInput").ap()
    out = nc.dram_tensor("out", [L, E], F32, kind="ExternalOutput").ap()

    with SplitDrainTileContext(nc) as tc:
        body(tc, xq, xk, xv, wall, out)
    _split_waits(nc)
    return nc


def _split_waits(nc, cap=1):
    """This walrus build rejects instructions carrying more than one sync
    wait.  Hoist excess waits onto same-engine NoOps inserted immediately
    before the instruction (engine program order enforces them first)."""
    ctr = 0
    for f in nc.m.functions:
        for bb in f.blocks:
            new = []
            for inst in bb.instructions:
                si = inst.sync_info
                waits = list(si.on_wait) if si is not None else []
                if len(waits) > cap:
                    for i in range(cap, len(waits), cap):
                        ctr += 1
                        nop = mybir.InstNoOp(name=f"I-waitnop-{ctr}")
                        nop.engine = inst.engine
                        nop.sync_info = mybir.SyncInfo(
                            on_wait=waits[i : i + cap], on_update=[]
                        )
                        nc.register_instruction(nop)
                        new.append(nop)
                    si.on_wait = waits[:cap]
                new.append(inst)
            bb.instructions = new
    return ctr


def body(tc, xq, xk, xv, wall, out):
    nc = tc.nc

    # ---- persistent SBUF tensors -------------------------------------
    persist = tc.alloc_tile_pool(name="persist", bufs=1)

    def ptile(name, shape):
        return persist.tile(shape, BF16, tag=name, name=name)

    w_sb = ptile("w_sb", [128, 8192])          # wk|wq|wv|wo
    qT_sb = ptile("qT_sb", [128, 2 * L])       # [dh-half-major, L]
    kT_sb = ptile("kT_sb", [128, 2 * L])
    v_sb = ptile("v_sb", [128, ST * DH])       # natural [S, dh], St-major
    oT_sb = ptile("oT_sb", [128, 2 * L])       # m(pair)-chunk-major
    x_sb = {}
    for nm in ("k", "q", "v"):
        for c in range(4):
            x_sb[(nm, c)] = ptile(f"x{nm}{c}", [128, CW])
    scratch = ptile("scratch", [128, 512])
    act_warm = ptile("act_warm", [128, 16])

    sc_pool = tc.alloc_tile_pool(name="sc", bufs=19)
    ou_pool = tc.alloc_tile_pool(name="ou", bufs=3)
    ps_sc = tc.alloc_tile_pool(name="ps_sc", bufs=2, space="PSUM")   # scores
    ps_sm = tc.alloc_tile_pool(name="ps_sm", bufs=2, space="PSUM")   # proj/outproj
    ps_o = tc.alloc_tile_pool(name="ps_o", bufs=2, space="PSUM")     # o_acc

    # ---- ACT table preload + PE warmup --------------------------------
    # The dummy activation is the FIRST scalar-engine instruction: it
    # forces the sigmoid ACT_TABLE_LOAD (~2.7us) during the initial DMA
    # wait.  It must precede any scalar dma_start: the HWDGE ring is ~4
    # transfers deep and a full ring blocks the issuing sequencer.
    nc.vector.memset(scratch[:], 0.0)
    nc.scalar.activation(act_warm[:], scratch[:, 0:16], AF.Sigmoid, scale=SCALE)
    wu_ps = ps_sc.tile([128, 1024], F32, tag="ps_sc", name="warmup_ps")
    for i in range(12):
        nc.tensor.matmul(
            wu_ps[:, :512], lhsT=scratch[:, :128], rhs=scratch[:],
            start=(i == 0), stop=(i == 11),
        )

    # ---- DMA schedule: wave-gated ------------------------------------
    # Three issue rings (sync/scalar HWDGE, gpsimd SWDGE).  All in-flight
    # transfers share HBM bandwidth ~fairly, so issuing everything up
    # front starves the critical first chunk.  Later waves are gated on
    # compute progress via a 1-element vector memset on the dst tile
    # (WAR dep): the dma descriptor generation then waits until the
    # anchor point in the vector program is reached.
    def q4(dst, src, base, ring):
        for j in range(2):
            ring.dma_start(
                dst[:, base + j * 1024 : base + (j + 1) * 1024],
                src[:, base + j * 1024 : base + (j + 1) * 1024],
            )

    def gated(ring, dst_tile, dst_sl, src, src_sl):
        nc.vector.memset(dst_tile[0:1, dst_sl.start : dst_sl.start + 1], 0.0)
        ring.dma_start(dst_tile[:, dst_sl], src[:, src_sl])

    # Wave 1 (ungated): chunk-0 k/q + wk/wq -> ~3MB in flight
    q4(x_sb[("k", 0)], xk, 0, nc.sync)       # e0-3 in two 256KB pieces
    q4(x_sb[("k", 0)], xk, 2048, nc.scalar)  # e4-7
    q4(w_sb, wall, WK_OFF, nc.gpsimd)        # wk halves: k-proj e0-3 can
    q4(w_sb, wall, WQ_OFF, nc.gpsimd)        # start before all of wk lands
    q4(x_sb[("q", 0)], xq, 0, nc.sync)
    q4(x_sb[("q", 0)], xq, 2048, nc.scalar)

    def wave2():  # anchor: k-c0 copies (~14us): xv0, wv, wo
        gated(nc.sync, x_sb[("v", 0)], slice(0, 2048), xv, slice(0, 2048))
        gated(nc.scalar, x_sb[("v", 0)], slice(2048, CW), xv, slice(2048, CW))
        gated(nc.gpsimd, w_sb, slice(WV_OFF, WV_OFF + 2048), wall, slice(WV_OFF, WV_OFF + 2048))
        gated(nc.gpsimd, w_sb, slice(WO_OFF, WO_OFF + 2048), wall, slice(WO_OFF, WO_OFF + 2048))

    def wave3():  # anchor: q-c0 copies (~17us): xk1, xv1
        gated(nc.sync, x_sb[("k", 1)], slice(0, CW), xk, slice(CW, 2 * CW))
        gated(nc.gpsimd, x_sb[("v", 1)], slice(0, 2048), xv, slice(CW, CW + 2048))
        gated(nc.gpsimd, x_sb[("v", 1)], slice(2048, CW), xv, slice(CW + 2048, 2 * CW))

    def wave4():  # anchor: v-c0 copies (~21us): xk2, xv2
        gated(nc.sync, x_sb[("k", 2)], slice(0, CW), xk, slice(2 * CW, 3 * CW))
        gated(nc.gpsimd, x_sb[("v", 2)], slice(0, 2048), xv, slice(2 * CW, 2 * CW + 2048))
        gated(nc.gpsimd, x_sb[("v", 2)], slice(2048, CW), xv, slice(2 * CW + 2048, 3 * CW))

    def wave5():  # anchor: v-c1 copies (~30us): xk3, xv3, xq1
        # NOTE: a gated dma must be EMITTED before any consumer of its
        # tile, or Tile never records the RAW dependency.  xq1's
        # consumers (q-proj c1) are emitted in sg2's scores phase, so it
        # must ride this wave, not a later one.
        gated(nc.sync, x_sb[("k", 3)], slice(0, CW), xk, slice(3 * CW, 4 * CW))
        gated(nc.gpsimd, x_sb[("v", 3)], slice(0, 2048), xv, slice(3 * CW, 3 * CW + 2048))
        gated(nc.gpsimd, x_sb[("v", 3)], slice(2048, CW), xv, slice(3 * CW + 2048, 4 * CW))
        gated(nc.gpsimd, x_sb[("q", 1)], slice(0, CW), xq, slice(CW, 2 * CW))

    def wave6():  # anchor: v-c2 copies (~39us): xq2, xq3
        gated(nc.sync, x_sb[("q", 2)], slice(0, CW), xq, slice(2 * CW, 3 * CW))
        gated(nc.gpsimd, x_sb[("q", 3)], slice(0, CW), xq, slice(3 * CW, 4 * CW))

    # ---- emission units ----------------------------------------------
    def kq_units(nm, c):
        """16 units: (mt, e) with mt outer; each unit = 1 matmul; psum
        [128,512] acc over e, copy to qT/kT at e==7."""
        woff = WK_OFF if nm == "k" else WQ_OFF
        dst = kT_sb if nm == "k" else qT_sb
        xt = x_sb[(nm, c)]
        for mt in range(2):
            box = {}
            for e in range(EC):
                def unit(nm=nm, c=c, mt=mt, e=e, box=box, woff=woff, dst=dst, xt=xt):
                    if e == 0:
                        box["acc"] = ps_sm.tile(
                            [128, 512], F32, tag="ps_sm", name=f"{nm}{c}m{mt}"
                        )
                    nc.tensor.matmul(
                        box["acc"][:],
                        lhsT=w_sb[:, woff + e * DH + mt * 128 : woff + e * DH + (mt + 1) * 128],
                        rhs=xt[:, e * 512 : (e + 1) * 512],
                        start=(e == 0),
                        stop=(e == EC - 1),
                    )
                    if e == EC - 1:
                        nc.vector.tensor_copy(
                            dst[:, mt * L + c * 512 : mt * L + (c + 1) * 512],
                            box["acc"][:],
                        )
                yield unit

    def v_units(c):
        """8 units: (st4, eh); vacc [128,256] acc over e, copy at eh==1."""
        xt = x_sb[("v", c)]
        for st4 in range(4):
            box = {}
            for eh in range(2):
                def unit(c=c, st4=st4, eh=eh, box=box, xt=xt):
                    st = c * 4 + st4
                    if eh == 0:
                        box["acc"] = ps_sm.tile(
                            [128, 256], F32, tag="ps_sm", name=f"v{st}"
                        )
                    for e in range(eh * 4, eh * 4 + 4):
                        nc.tensor.matmul(
                            box["acc"][:],
                            lhsT=xt[:, e * 512 + st4 * 128 : e * 512 + (st4 + 1) * 128],
                            rhs=w_sb[:, WV_OFF + e * DH : WV_OFF + (e + 1) * DH],
                            start=(e == 0),
                            stop=(e == EC - 1),
                        )
                    if eh == 1:
                        nc.vector.tensor_copy(
                            v_sb[:, st * DH : (st + 1) * DH], box["acc"][:]
                        )
                yield unit

    sc_map = {}

    def sc_step(lc, st):
        """Both pairs' scores for one (lc, st): 4 matmuls emitted
        back-to-back alternating row halves (p0s0, p0s1, p1s0, p1s1) so
        each matmul is row-disjoint from the in-flight one and they
        chain-overlap in the PE array."""
        for pair in range(2):
            ps = ps_sc.tile([128, 1024], F32, tag="ps_sc", name=f"scps{lc}_{st}_{pair}")
            for sub in range(2):
                nc.tensor.matmul(
                    ps[:, sub * 512 : (sub + 1) * 512],
                    lhsT=kT_sb[
                        sub * 64 : (sub + 1) * 64,
                        pair * L + st * 128 : pair * L + (st + 1) * 128,
                    ],
                    rhs=qT_sb[
                        sub * 64 : (sub + 1) * 64,
                        pair * L + lc * 512 : pair * L + (lc + 1) * 512,
                    ],
                    start=True,
                    stop=True,
                    tile_position=(sub * 64, 0),
                )
            sc = sc_pool.tile([128, 1024], BF16, tag="sc", name=f"sc{lc}_{st}_{pair}")
            nc.scalar.activation(sc[:], ps[:], AF.Sigmoid, scale=SCALE)
            sc_map[(lc, st, pair)] = sc

    o_acc_cur = [None, None]

    def ao_pair(lc, st, pair):
        if st == 0:
            o_acc_cur[pair] = ps_o.tile(
                [128, 512], F32, tag="ps_o", name=f"oacc{lc}_{pair}"
            )
        sc = sc_map.pop((lc, st, pair))
        for sub in range(2):
            h = pair * 2 + sub
            nc.tensor.matmul(
                o_acc_cur[pair][sub * 64 : (sub + 1) * 64, :],
                lhsT=v_sb[:, st * DH + h * 64 : st * DH + (h + 1) * 64],
                rhs=sc[:, sub * 512 : (sub + 1) * 512],
                start=(st == 0),
                stop=(st == ST - 1),
                tile_position=(0, sub * 64),
                # Sim's psum-group bookkeeping mis-addresses
                # partition-offset groups; has_written is per-element
                # on HW and the two halves are disjoint.
                skip_group_check=True,
            )
        if st == ST - 1:
            # copy each pair's o_acc as soon as ITS last matmul is done,
            # so the next lc's first ao pair doesn't stall on the pool
            nc.vector.tensor_copy(
                oT_sb[:, pair * L + lc * 512 : (pair * L + (lc + 1) * 512)],
                o_acc_cur[pair][:],
            )
            if pair == 1:
                filler.extend(outproj_units(lc))

    def outproj_units(lc):
        for lt in range(4):
            def unit(lc=lc, lt=lt):
                lg = lc * 512 + lt * 128
                ot = ou_pool.tile([128, E], F32, tag="ou", name=f"ot{lc}_{lt}")
                # psum from ps_sm (NOT ps_sc): holding a scores-pool buf
                # for 4 matmuls starves the sigmoid pipeline.  m outer:
                # each oT weight tile loads once, used for both ec halves.
                pss = [
                    ps_sm.tile([128, 512], F32, tag="ps_sm", name=f"ops{lc}_{lt}_{ec}")
                    for ec in range(2)
                ]
                for m in range(2):
                    for ec in range(2):
                        nc.tensor.matmul(
                            pss[ec][:],
                            lhsT=oT_sb[:, m * L + lg : m * L + lg + 128],
                            rhs=w_sb[:, WO_OFF + m * E + ec * 512 : WO_OFF + m * E + (ec + 1) * 512],
                            start=(m == 0),
                            stop=(m == 1),
                        )
                for ec in range(2):
                    # last lc: ACT is done with sigmoids -- use it for
                    # half the copies so the tail pipeline is 2-wide
                    if lc == 3 and ec == 1:
                        nc.scalar.copy(ot[:, ec * 512 : (ec + 1) * 512], pss[ec][:])
                    else:
                        nc.vector.tensor_copy(ot[:, ec * 512 : (ec + 1) * 512], pss[ec][:])
                if lc == 3:
                    # split final stores across two rings to shrink the tail
                    nc.sync.dma_start(out[lg : lg + 128, 0:512], ot[:, 0:512])
                    nc.gpsimd.dma_start(out[lg : lg + 128, 512:1024], ot[:, 512:1024])
                else:
                    eng = [nc.gpsimd, nc.sync, nc.scalar][(lc * 4 + lt) % 3]
                    eng.dma_start(out[lg : lg + 128, :], ot[:])
            yield unit

    filler = []

    def pop_filler(n=1):
        for _ in range(min(n, len(filler))):
            filler.pop(0)()

    def flush_filler():
        while filler:
            filler.pop(0)()

    def interleave(units_a, units_b):
        """Alternate: one a, one b, until both exhausted."""
        a, b = list(units_a), list(units_b)
        while a or b:
            if a:
                a.pop(0)()
            if b:
                b.pop(0)()

    # ---- lc0: bootstrap pipeline -------------------------------------
    for i, u in enumerate(kq_units("k", 0)):
        u()
        if i == 7:      # k-c0 mt0 copy just emitted
            wave2()
    for i, u in enumerate(kq_units("q", 0)):
        u()
        if i == 7:
            wave3()
    sc_step(0, 0)
    vu = list(v_units(0))
    for st in (1, 2, 3):
        vu.pop(0)()
        vu.pop(0)()
        sc_step(0, st)
    vu.pop(0)()
    vu.pop(0)()
    wave4()         # anchor: v-c0 copies
    # ao sg0 interleaved with k c1 (2 kq units per ao pair)
    ku = list(kq_units("k", 1))
    for st in range(4):
        for p in range(2):
            ku.pop(0)()
            ku.pop(0)()
            ao_pair(0, st, p)

    for sg in (1, 2, 3):
        # scores of sg interleaved with v c(sg) (+ q c1 during sg2)
        vu = list(v_units(sg))
        extra = list(kq_units("q", 1)) if sg == 2 else []
        for st4 in range(4):
            vu.pop(0)()
            vu.pop(0)()
            for _ in range(4):
                if extra:
                    extra.pop(0)()
            sc_step(0, sg * 4 + st4)
        [None, wave5, wave6, lambda: None][sg]()   # anchor: v-c(sg) copies
        # ao of sg interleaved with k c(sg+1); from sg2 on, run the
        # scores pipeline ahead into lc1 (shifts sigmoid work into lc0's
        # ACT-idle window; q-proj c1 completed in sg2's scores phase)
        nxt = list(kq_units("k", sg + 1)) if sg < 3 else []
        lead_sc = {2: [(1, 0), (1, 1)], 3: [(1, s) for s in range(2, 8)]}.get(sg, [])
        for st4 in range(4):
            for p in range(2):
                if nxt:
                    nxt.pop(0)()
                    nxt.pop(0)()
                if lead_sc:
                    sc_step(*lead_sc.pop(0))
                ao_pair(0, sg * 4 + st4, p)

    # ---- lc 1..3: steady pipeline ------------------------------------
    # sc stream runs LEAD steps ahead of the ao stream.  q-proj chunks
    # 2/3 are emitted as ATOMIC 8-unit (one mt half: psum acc + copy)
    # lumps: a partially-accumulated ps_sm tile interleaved with outproj
    # allocations from the same pool would deadlock the PE queue.
    LEAD = 8
    steps2 = [(lc, st) for lc in (1, 2, 3) for st in range(16)]
    n = len(steps2)
    q2u, q3u = list(kq_units("q", 2)), list(kq_units("q", 3))
    q_sched = {1: q2u[0:8], 5: q2u[8:16], 17: q3u[0:8], 21: q3u[8:16]}
    # outproj filler is rationed (one unit every 4th step) so some of it
    # is left to fill the PE during the last, otherwise attention-only lc
    op_sched = {2, 6, 10, 14, 18, 22, 26, 30, 34, 36, 38, 40, 42, 44, 46, 47}

    for j in range(n):
        tgt = j + LEAD
        if tgt < n:
            sc_step(*steps2[tgt])
        if j in q_sched:
            for u in q_sched.pop(j):
                u()
        lc, st = steps2[j]
        if j in op_sched:
            pop_filler(1)
        ao_pair(lc, st, 0)
        ao_pair(lc, st, 1)
    flush_filler()

    # release pools in reverse allocation (stack) order
    for pool in (ps_o, ps_sm, ps_sc, ou_pool, sc_pool, persist):
        pool.release()


_NC_CACHE = None


def _get_nc():
    global _NC_CACHE
    if _NC_CACHE is None:
        _NC_CACHE = build_nc()
    return _NC_CACHE


def _chunk_major(xT):
    """[E=1024, L=2048] -> [128, 4*4096]: out[p, c*4096 + e*512 + l] =
    xT[e*128+p, c*512+l]."""
    return np.ascontiguousarray(
        xT.reshape(8, 128, 4, 512).transpose(1, 2, 0, 3).reshape(128, 4 * CW)
    )


def _echunk_major(wT, nchunk, width):
    """[nchunk*128, width] -> [128, nchunk*width]."""
    return wT.reshape(nchunk, 128, width).transpose(1, 0, 2).reshape(128, nchunk * width)


def _prep_in_maps(query, key, value, Wq, Wk, Wv, Wo):
    B = query.shape[0]
    bf = ml_dtypes.bfloat16
    xprep = {}
    for b in range(B):
        for nm, src in (("q", query), ("k", key), ("v", value)):
            xprep[(nm, b)] = _chunk_major(
                np.ascontiguousarray(src[b].T).astype(bf)
            )
    in_maps = []
    for c in range(N_CORES):
        b, g = c // 4, c % 4
        hs = slice(g * DH, (g + 1) * DH)
        wparts = [
            _echunk_major(np.ascontiguousarray(Wk[hs, :].T).astype(bf), 8, DH),
            _echunk_major(np.ascontiguousarray(Wq[hs, :].T).astype(bf), 8, DH),
            _echunk_major(np.ascontiguousarray(Wv[hs, :].T).astype(bf), 8, DH),
            _echunk_major(np.ascontiguousarray(Wo[:, hs].T).astype(bf), 2, E),
        ]
        in_maps.append(
            {
                "xq": xprep[("q", b)],
                "xk": xprep[("k", b)],
                "xv": xprep[("v", b)],
                "wall": np.ascontiguousarray(np.concatenate(wparts, axis=1)),
            }
        )
    return in_maps


LAST_RESULTS = None


def run_sharded(query, key, value, Wq, Wk, Wv, Wo, trace=False, tmpdir=None):
    global LAST_RESULTS
    if trace:
        # Shim the missing antenv.axon_hooks so NTFF tracing works under axon.
        import sys
        import types

        try:
            import antenv.axon_hooks  # noqa: F401
        except ImportError:
            from trn_agent_boot.trn_boot import _ntff_profile_via_ctypes

            _mod = types.ModuleType("antenv.axon_hooks")
            _hook = _ntff_profile_via_ctypes("/opt/axon/libaxon_pjrt.so")
            _mod.get_axon_ntff_profile_hook = lambda: _hook
            sys.modules["antenv.axon_hooks"] = _mod
        bass_utils.upload_artifacts = lambda tmpdir: tmpdir

    nc = _get_nc()
    in_maps = _prep_in_maps(query, key, value, Wq, Wk, Wv, Wo)
    res = bass_utils.run_bass_kernel_spmd(
        nc, in_maps, core_ids=list(range(N_CORES)), trace=trace, tmpdir=tmpdir
    )
    LAST_RESULTS = res
    B = query.shape[0]
    full = np.zeros((B, L, E), dtype=np.float32)
    for c in range(N_CORES):
        full[c // 4] += res.results[c]["out"]
    return full


def kernel(query, key, value, Wq, Wk, Wv, Wo):
    return run_sharded(query, key, value, Wq, Wk, Wv, Wo, trace=False)
